# revision 2
# baseline (speedup 1.0000x reference)
"""Multi-head causal attention (B=2, C=2048, E=1024, H=16) on 8 NeuronCores.

Sharding: tensor-parallel over (batch, head-group): core = b*4 + g handles
batch b and heads [4g, 4g+4). Host sums the 4 partials per batch and adds bo.

v2 schedule (vs v1 baseline at ~161.5us):
  - DMA issuance split across the two HWDGE queues (sync + scalar) so input
    data starts flowing right after the NEFF preamble instead of ~9us in.
  - K projection runs chunk-outer (8 PSUM accumulators across the big/ctx/pj
    tags) so the PE consumes xT e-chunks as they arrive off DMA.
  - Projections (Q, V) and the output projection are emitted as fine-grained
    "filler" micro-steps inside the attention chunk loop, absorbing the
    scalar-engine exp deficit (exp is ~1.15us/chunk vs 0.85us of PE work).
  - Attention pair 1 runs q-tiles descending with wo(tt) fillers so the tail
    exposes only ~wo(0).
  - Norm-stage copies on gpsimd, wo copies split vector/gpsimd: keeps vector
    under the PE roofline during attention.

Dataflow identical to v1 (fully transposed, fp16 matmuls, no max-subtraction;
|scores/32| < ~2.5 so exp is safe).
"""
import numpy as np

import concourse.bass as bass
import concourse.tile as tile
from concourse import bacc, mybir
from concourse.bass_utils import run_bass_kernel_spmd

F16 = mybir.dt.float16
F32 = mybir.dt.float32

B, C, E, H = 2, 2048, 1024, 16
NH = 4              # heads per core
D = 64              # head dim
FS = NH * D         # 256 features per core
EC = E // 128       # 8 e-chunks
QT = 512            # q tile size
NQ = C // QT        # 4 q tiles
SCALE = 1.0 / np.sqrt(np.float32(E))  # note: module scales by sqrt(E)

_CACHED_NC = None


def build():
    nc = bacc.Bacc("TRN2", target_bir_lowering=False, debug=False, num_devices=8)
    xT = nc.dram_tensor("xT", [E, C], F16, kind="ExternalInput")
    wk = nc.dram_tensor("wk", [128, EC, FS], F16, kind="ExternalInput")
    # packA = wq | wv  [128, 2, EC, FS]; packB = msk | wo  [128, 4096]
    packA = nc.dram_tensor("packA", [128, 2, EC, FS], F16, kind="ExternalInput")
    packB = nc.dram_tensor("packB", [128, 4096], F16, kind="ExternalInput")
    out = nc.dram_tensor("out", [E, C], F16, kind="ExternalOutput")  # out^T

    with tile.TileContext(nc) as tc:
        with tc.tile_pool(name="const", bufs=1) as cp, \
             tc.tile_pool(name="work", bufs=1) as wp, \
             tc.tile_pool(name="ps", bufs=1, space="PSUM") as ps:
            # ---- resident SBUF tensors ----
            xT_sb = cp.tile([128, EC, C], F16)
            wk_sb = cp.tile([128, EC, FS], F16)
            packA_sb = cp.tile([128, 2, EC, FS], F16)
            packB_sb = cp.tile([128, 4096], F16)
            wq_sb = packA_sb[:, 0]
            wv_sb = packA_sb[:, 1]
            msk_sb = packB_sb[:, 0:2048].rearrange("p (r q) -> p r q", r=4)
            wo_sb = packB_sb[:, 2048:4096].rearrange("p (g e) -> p g e", g=2)
            qt_sb = cp.tile([128, 2, C], F16)
            kt_sb = cp.tile([128, 2, C], F16)
            # V_aug per (t-chunk, head): [128 k, 64 d | 64 ones].  The 64
            # ones-columns make the ctx matmul emit the softmax denominator
            # l[q] replicated across PSUM partitions 64..127 — the normalizer
            # broadcast comes free out of the PE, killing the
            # stage/gather/partition_broadcast chain of the v1 design.
            v_sb = cp.tile([128, C // 128, NH * 128], F16)
            ctxt_sb = cp.tile([128, 2, C], F16)

            # ---- input DMAs, single sync queue, strict priority order ----
            nc.sync.dma_start(wk_sb[:], wk[:])
            for c in range(EC):
                nc.sync.dma_start(xT_sb[:, c, :], xT[c * 128:(c + 1) * 128, :])
            nc.sync.dma_start(packA_sb[:], packA[:])
            nc.sync.dma_start(packB_sb[:], packB[:])

            # ones columns of v_sb (cols D..127 of each head slot); V-proj
            # copies only touch cols 0..D-1 so these survive.  gpsimd: the
            # 4us memset would otherwise block the vector engine's early work
            ones_cols = v_sb[:].rearrange(
                "p t (h x) -> p t h x", h=NH)[:, :, :, D:128]
            nc.gpsimd.memset(ones_cols, 1.0)

            # ---- PE warm-up while first DMAs land ----
            wu = wp.tile([128, QT], F16, tag="wu", bufs=1)
            nc.vector.memset(wu[:], 0.5)
            for i in range(10):
                wups = ps.tile([128, QT], F32, tag="pj", bufs=2,
                               name=f"wups_{i}")
                nc.tensor.matmul(wups[:], lhsT=wu[:, 0:128], rhs=wu[:],
                                 start=True, stop=True, skip_group_check=True)

            # ---- K projection, chunk-outer over xT e-chunks in DMA-arrival
            # order; 8 live PSUM accumulators mapped onto the big/ctx/pj tags
            kb = [ps.tile([128, 2 * QT], F32, tag="big", bufs=2,
                          name=f"kb{i}") for i in range(2)]
            kc = [ps.tile([128, QT], F32, tag="ctx", bufs=2,
                          name=f"kc{i}") for i in range(2)]
            kp = [ps.tile([128, QT], F32, tag="pj", bufs=2,
                          name=f"kp{i}") for i in range(2)]
            kslot = {(0, 0): kb[0][:, 0:QT], (0, 1): kb[0][:, QT:2 * QT],
                     (0, 2): kb[1][:, 0:QT], (0, 3): kb[1][:, QT:2 * QT],
                     (1, 0): kc[0][:], (1, 1): kc[1][:],
                     (1, 2): kp[0][:], (1, 3): kp[1][:]}
            korder = [(1, 0), (1, 1), (0, 0), (0, 1), (0, 2), (0, 3),
                      (1, 2), (1, 3)]
            arrival = [0, 1, 2, 3, 4, 5, 6, 7]
            for ci, c in enumerate(arrival):
                for (g2, j) in korder:
                    nc.tensor.matmul(
                        kslot[(g2, j)],
                        lhsT=wk_sb[:, c, 128 * g2:128 * (g2 + 1)],
                        rhs=xT_sb[:, c, QT * j:QT * (j + 1)],
                        start=(ci == 0), stop=(ci == 7),
                        skip_group_check=True,
                    )
            # copy-outs: ctx slots first (they gate Q00/V0), split vec/scalar
            nc.vector.tensor_copy(kt_sb[:, 1, 0:QT], kc[0][:])
            nc.scalar.activation(kt_sb[:, 1, QT:2 * QT], kc[1][:],
                                 mybir.ActivationFunctionType.Copy)
            nc.vector.tensor_copy(kt_sb[:, 0, 0:QT], kb[0][:, 0:QT])
            nc.scalar.activation(kt_sb[:, 0, QT:2 * QT], kb[0][:, QT:2 * QT],
                                 mybir.ActivationFunctionType.Copy)
            nc.vector.tensor_copy(kt_sb[:, 0, 2 * QT:3 * QT], kb[1][:, 0:QT])
            nc.scalar.activation(kt_sb[:, 0, 3 * QT:4 * QT],
                                 kb[1][:, QT:2 * QT],
                                 mybir.ActivationFunctionType.Copy)
            nc.vector.tensor_copy(kt_sb[:, 1, 2 * QT:3 * QT], kp[0][:])
            nc.scalar.activation(kt_sb[:, 1, 3 * QT:4 * QT], kp[1][:],
                                 mybir.ActivationFunctionType.Copy)

            # ---- filler machinery: proj/wo units as generators ----
            def gen_proj_q(g2, j, tag=None):
                pp = ps.tile([128, QT], F32, tag=tag or cur["fill"], bufs=2,
                             name=f"ppq_{g2}_{j}")
                for c in range(EC):
                    nc.tensor.matmul(
                        pp[:],
                        lhsT=wq_sb[:, c, 128 * g2:128 * (g2 + 1)],
                        rhs=xT_sb[:, c, QT * j:QT * (j + 1)],
                        start=(c == 0), stop=(c == EC - 1),
                    )
                    yield
                nc.vector.tensor_copy(qt_sb[:, g2, QT * j:QT * (j + 1)], pp[:])

            def gen_proj_v(t, tag=None):
                pp = ps.tile([128, FS], F32, tag=tag or cur["fill"], bufs=2,
                             name=f"ppv_{t}")
                for c in range(EC):
                    nc.tensor.matmul(
                        pp[:],
                        lhsT=xT_sb[:, c, 128 * t:128 * (t + 1)],
                        rhs=wv_sb[:, c, :],
                        start=(c == 0), stop=(c == EC - 1),
                    )
                    if c % 2 == 1:
                        yield
                nc.vector.tensor_copy(
                    v_sb[:, t, :].rearrange("p (h x) -> p h x", h=NH)[:, :, 0:D],
                    pp[:].rearrange("p (h d) -> p h d", h=NH),
                )

            def gen_wo(tt, tags=None, alt=False, per_ec=False):
                # alt: route half the PSUM->SBUF copies to the scalar engine
                # (only safe once its exp load has tapered off).
                # per_ec: one output DMA per e-chunk so the final unit's
                # output drains while its later matmuls still run.
                for ep in range(4):
                    ot = wp.tile([128, 2, QT], F16, tag="ot", bufs=4,
                                 name=f"ot_{tt}_{ep}")
                    for k in range(2):
                        ec = 2 * ep + k
                        tg = tags[ec % len(tags)] if tags else cur["fill"]
                        pp = ps.tile([128, QT], F32, tag=tg,
                                     bufs=2, name=f"ppwo_{tt}_{ec}")
                        for g2 in range(2):
                            nc.tensor.matmul(
                                pp[:],
                                lhsT=wo_sb[:, g2, 128 * ec:128 * (ec + 1)],
                                rhs=ctxt_sb[:, g2, QT * tt:QT * (tt + 1)],
                                start=(g2 == 0), stop=(g2 == 1),
                            )
                            yield
                        if alt and k == 1:
                            nc.scalar.activation(
                                ot[:, k, :], pp[:],
                                mybir.ActivationFunctionType.Copy)
                        else:
                            nc.vector.tensor_copy(ot[:, k, :], pp[:])
                        yield
                        if per_ec:
                            nc.sync.dma_start(
                                out[128 * ec:128 * (ec + 1),
                                    QT * tt:QT * (tt + 1)], ot[:, k, :])
                    if not per_ec:
                        nc.sync.dma_start(
                            out[256 * ep:256 * (ep + 1),
                                QT * tt:QT * (tt + 1)]
                            .rearrange("(k p) q -> p k q", k=2),
                            ot[:])

            pending = {}      # name -> generator
            order = []        # emission order of pending units
            started = set()   # units with >=1 emitted step
            cur = {"fill": "pj"}   # PSUM tag for filler units

            def add_unit(name, gen):
                pending[name] = gen
                order.append(name)

            def finish_unit(name):
                g = pending.pop(name, None)
                if g is None:
                    return
                order.remove(name)
                started.discard(name)
                for _ in g:
                    pass

            def finish_started():
                # a half-emitted filler holds a PSUM slot whose copy-out is
                # not yet emitted; attention accumulators rotating onto the
                # same tag would deadlock the PE stream. Close it out first.
                for name in list(started):
                    finish_unit(name)

            def fill(steps):
                while steps > 0 and order:
                    name = order[0]
                    try:
                        next(pending[name])
                        started.add(name)
                        steps -= 1
                    except StopIteration:
                        pending.pop(name)
                        order.pop(0)
                        started.discard(name)

            # ---- attention ----
            def emit_scores(heads, j, c):
                q0 = 128 * (c - 4 * j) if c >= 4 * j else 0
                st = ps.tile([128, 2 * QT], F32, tag="big", bufs=2,
                             name=f"st_{heads[0]}_{j}_{c}")
                for i, h in enumerate(heads):
                    g2, po = h // 2, 64 * (h % 2)
                    nc.tensor.matmul(
                        st[:, QT * i + q0:QT * (i + 1)],
                        lhsT=kt_sb[po:po + 64, g2, 128 * c:128 * (c + 1)],
                        rhs=qt_sb[po:po + 64, g2, QT * j + q0:QT * (j + 1)],
                        start=True, stop=True,
                    )
                pt = wp.tile([128, 2 * QT], F16, tag="pt", bufs=6)
                st3 = st[:].rearrange("p (b q) -> p b q", b=2)[:, :, q0:QT]
                pt3 = pt[:].rearrange("p (b q) -> p b q", b=2)[:, :, q0:QT]
                nc.scalar.activation(
                    pt3, st3, mybir.ActivationFunctionType.Exp, scale=SCALE)
                if c >= 4 * j:
                    # causal mask, in place, on the 128-wide diagonal strip
                    # (vector: it gates the ctx matmul, gpsimd is too slow)
                    dm = pt[:].rearrange("p (b q) -> p b q", b=2)[
                        :, :, q0:q0 + 128]
                    nc.vector.tensor_mul(
                        dm, dm, msk_sb[:, c - 4 * j, q0:q0 + 128]
                        .unsqueeze(1).broadcast_to([128, 2, 128]))
                return pt

            unit_idx = [0]

            def attention(pair, j, steps_per_chunk, add_at=()):
                heads = (2 * pair, 2 * pair + 1)
                finish_unit(f"Q{pair}{j}")
                finish_started()
                # alternate accumulator/filler tags so this unit's ctx PSUM
                # never rotates onto the previous unit's (its slow normalize
                # chain would stall our first chunks)
                acc = "ctx" if unit_idx[0] % 2 == 0 else "pj"
                cur["fill"] = "pj" if acc == "ctx" else "ctx"
                unit_idx[0] += 1
                nk = 4 * (j + 1)
                ctx_ps = {h: ps.tile([128, QT], F32, tag=acc, bufs=2,
                                     name=f"ctx_{pair}_{j}_{h}")
                          for h in heads}
                pts = {}
                depth = min(2, nk)
                for c in range(depth):
                    pts[c] = emit_scores(heads, j, c)
                for c in range(nk):
                    for at, name, gen in add_at:
                        if at == c:
                            add_unit(name, gen())
                    if c + depth < nk:
                        pts[c + depth] = emit_scores(heads, j, c + depth)
                    finish_unit(f"V{c}")
                    if c + 1 < nk:
                        finish_unit(f"V{c + 1}")   # lookahead: its copy-out
                        # must not gate the next chunk's ctx matmul
                    pt = pts.pop(c)
                    q0 = 128 * (c - 4 * j) if c >= 4 * j else 0
                    for i, h in enumerate(heads):
                        nc.tensor.matmul(
                            ctx_ps[h][:, q0:QT],
                            lhsT=v_sb[:, c, 128 * h:128 * (h + 1)],
                            rhs=pt[:, QT * i + q0:QT * (i + 1)],
                            start=(c == 0), stop=(c == nk - 1),
                        )
                    fill(steps_per_chunk)
                # normalize: ctx_ps rows 0..63 = ctx^T, rows 64..127 = l[q]
                # replicated. Stage l through SBUF (reciprocal_approx_fast
                # cannot read PSUM), then scale directly out of PSUM.
                for i, h in enumerate(heads):
                    g2, po = h // 2, 64 * (h % 2)
                    ls = wp.tile([64, QT], F32, tag="ls", bufs=4,
                                 name=f"ls_{pair}_{j}_{h}")
                    nc.vector.tensor_copy(ls[:], ctx_ps[h][64:128, :])
                    rc = wp.tile([64, QT], F32, tag="rc", bufs=4,
                                 name=f"rc_{pair}_{j}_{h}")
                    nc.vector.reciprocal_approx_fast(rc[:], ls[:])
                    nc.vector.tensor_mul(
                        ctxt_sb[po:po + 64, g2, QT * j:QT * (j + 1)],
                        ctx_ps[h][0:64, :], rc[:])

            # ---- phase 2: first Q tile + first V tiles.  All on "pj": the
            # first attention's accumulators then rotate from the K-phase
            # kc tiles, whose copy-outs complete early.
            for _ in gen_proj_q(0, 0, "pj"):
                pass
            for t in (0, 1, 2, 3):
                for _ in gen_proj_v(t, "pj"):
                    pass

            # ---- filler queue (PSUM tag resolved lazily per unit from
            # cur["fill"], which attention() flips each unit) ----
            add_unit("Q01", gen_proj_q(0, 1))
            for t in (4, 5, 6, 7):
                add_unit(f"V{t}", gen_proj_v(t))
            add_unit("Q02", gen_proj_q(0, 2))
            for t in (8, 9, 10, 11):
                add_unit(f"V{t}", gen_proj_v(t))
            add_unit("Q03", gen_proj_q(0, 3))
            for t in (12, 13, 14, 15):
                add_unit(f"V{t}", gen_proj_v(t))
            add_unit("Q13", gen_proj_q(1, 3))
            add_unit("Q12", gen_proj_q(1, 2))
            add_unit("Q11", gen_proj_q(1, 1))
            add_unit("Q10", gen_proj_q(1, 0))

            # ---- pair 0 ascending, pair 1 descending with wo fillers.
            # Filler steps tuned so the late attention units (largest
            # scalar-exp deficit, fewest remaining units) are never starved.
            # wo(tt) is queued a few chunks into the following attention so
            # its first matmul lands after ctxt(tt) is actually written.
            attention(0, 0, 1)
            attention(0, 1, 1)
            attention(0, 2, 2)
            attention(0, 3, 2)
            attention(1, 3, 3)
            attention(1, 2, 2, add_at=((2, "wo3", lambda: gen_wo(3)),))
            attention(1, 1, 3, add_at=((2, "wo2", lambda: gen_wo(2)),))
            attention(1, 0, 6,
                      add_at=((1, "wo1", lambda: gen_wo(1, alt=True)),))
            # post-attention: big/ctx PSUM banks are free, rotate across all
            add_unit("wo0", gen_wo(0, tags=("pj", "ctx", "big"), alt=True,
                                   per_ec=True))
            fill(10 ** 9)   # drain remaining wo work
    nc.compile()
    return nc


def _causal_masks():
    # [128 k-partitions, 4 r, QT q] layout for contiguous DMA
    k = np.arange(128)[:, None]
    q = np.arange(QT)[None, :]
    m = np.stack([(k + 128 * r <= q) for r in range(4)])      # [4, 128, QT]
    return np.ascontiguousarray(m.transpose(1, 0, 2)).astype(np.float16)


def _w_in(w):
    # [E, FS] -> [128 p, EC chunks, FS] (e = c*128 + p)
    return np.ascontiguousarray(
        w.reshape(EC, 128, FS).transpose(1, 0, 2)).astype(np.float16)


def _wo_in(w):
    # [FS, E] -> [128 p, 2 g, E] (f = g*128 + p)
    return np.ascontiguousarray(
        w.reshape(2, 128, E).transpose(1, 0, 2)).astype(np.float16)


def make_in_maps(x, Wq, Wk, Wv, Wo, bo):
    msk = _causal_masks()                                     # [128, 4, 512]
    in_maps = []
    for b in range(B):
        xT_h = np.ascontiguousarray(np.asarray(x, np.float32)[b].T
                                    ).astype(np.float16)
        for g in range(4):
            s = slice(g * FS, (g + 1) * FS)
            wq_l = _w_in(np.asarray(Wq, np.float32)[:, s])    # [128, 8, 256]
            wv_l = _w_in(np.asarray(Wv, np.float32)[:, s])
            wo_l = _wo_in(np.ascontiguousarray(
                np.asarray(Wo, np.float32)[s, :]))            # [128, 2, 1024]
            packA = np.ascontiguousarray(
                np.stack([wq_l, wv_l], axis=1))               # [128, 2, 8, 256]
            packB = np.ascontiguousarray(np.concatenate(
                [msk.reshape(128, 2048), wo_l.reshape(128, 2048)], axis=1))
            in_maps.append({
                "xT": xT_h,
                "wk": _w_in(np.asarray(Wk, np.float32)[:, s]),
                "packA": packA,
                "packB": packB,
            })
    return in_maps


def kernel(x, Wq, Wk, Wv, Wo, bo):
    global _CACHED_NC
    bo = np.asarray(bo, np.float32)

    if _CACHED_NC is None:
        _CACHED_NC = build()
    nc = _CACHED_NC

    in_maps = make_in_maps(x, Wq, Wk, Wv, Wo, bo)
    res = run_bass_kernel_spmd(nc, in_maps, core_ids=list(range(8)))

    out = np.empty((B, C, E), np.float32)
    for b in range(B):
        acc = res.results[b * 4 + 0]["out"].astype(np.float32)
        for g in range(1, 4):
            acc += res.results[b * 4 + g]["out"]
        out[b] = acc.T + bo          # kernel emits out^T
    return out
